# revision 14
# baseline (speedup 1.0000x reference)
"""GAT encoder (2-layer, PyG-style) on 8 Trainium2 NeuronCores.

v2 architecture — "lane layout" edge processing, no per-tile loops:
  - Nodes sharded by range across 8 cores (6250 own/core). Per core, TWO
    node->slot bijections: sigma_d (sorted by in-degree) for all node arrays
    and the dst-grouped edge layout; sigma_s (sorted by out-degree) for the
    src-grouped edge layout of layer-2 pass 2.
  - Edge arrays are [128, W]: edge (s->d) sits at partition sd(d)%128,
    in the column range of block sd(d)//128 (degree-sorted packing makes
    W ~= E/128 with ~98% density). Per-dst-node values broadcast via ONE
    wide matmul (block one-hot), per-dst segment sums via 49 free-dim
    reduces. Zero per-tile one-hot/transpose machinery.
  - Per-edge values keyed by the *other* endpoint are fetched with
    dma_gather (SWDGE, 256B rows, one instruction per chunk) from
    AllGathered tables, then selected with host-baked bf16 one-hot masks:
    gpsimd mult + DVE inner reduce.
  - Layer 2: pass 1 (dst layout) computes softmax denominators; AllGather
    (a_dst2, 1/denom) pairs; pass 2 (src layout) computes per-edge coef and
    reduces c[s] = sum coef by src. Final P = sum_n c[n] h2[n], AllReduce.
"""

import sys
import numpy as np

sys.path.insert(0, "/opt/trn_rl_repo")

import concourse.bass as bass
import concourse.bacc as bacc
import concourse.mybir as mybir
import concourse.tile as tile
from concourse.bass_utils import run_bass_kernel_spmd

P = 128
NCORES = 8
N = 50000
NOWN = N // NCORES          # 6250
NBL = 49                    # blocks per core (49*128 = 6272 slots)
NSLOT = NBL * P
NEG = 0.2
CHUNKS = 16

F32 = mybir.dt.float32
BF16 = mybir.dt.bfloat16
I16 = mybir.dt.int16

_CACHE = {}


# ----------------------------------------------------------------------------
# Host-side prep: pure index/permutation work (gathers of x, one-hot masks).
# ----------------------------------------------------------------------------

def _wrap_idx(idxmat):
    """[128, W] row-index matrix -> dma_gather idx tile [128, 128*W//16] i16.
    Logical idx order i = w*128 + p; wrapped in 16 partitions, replicated
    across the 8 Q7 cores."""
    ni = idxmat.size
    lst = idxmat.T.reshape(ni)                      # lst[i] = idxmat[i%128, i//128]
    sb = lst.reshape(ni // 16, 16).T                # [16, ni/16]
    return np.ascontiguousarray(np.tile(sb, (8, 1)).astype(np.int16))


def host_prep(x, edge_index):
    src = np.concatenate([edge_index[0], np.arange(N)]).astype(np.int64)
    dst = np.concatenate([edge_index[1], np.arange(N)]).astype(np.int64)

    # --- per-core slot assignments + global block widths ---
    sd_map = np.zeros(N, np.int64)    # node -> sigma_d slot (local)
    ss_map = np.zeros(N, np.int64)    # node -> sigma_s slot (local)
    od_all, os_all = [], []
    W1_r = np.zeros(NBL, np.int64)
    W2_r = np.zeros(NBL, np.int64)
    for c in range(NCORES):
        lo = c * NOWN
        dd = np.bincount(dst[(dst // NOWN) == c] - lo, minlength=NOWN)
        sdeg = np.bincount(src[(src // NOWN) == c] - lo, minlength=NOWN)
        od = np.argsort(-dd, kind="stable")
        os_ = np.argsort(-sdeg, kind="stable")
        rd = np.empty(NOWN, np.int64); rd[od] = np.arange(NOWN)
        rs = np.empty(NOWN, np.int64); rs[os_] = np.arange(NOWN)
        sd_map[lo:lo + NOWN] = rd
        ss_map[lo:lo + NOWN] = rs
        od_all.append(od); os_all.append(os_)
        dds = np.concatenate([dd[od], np.zeros(NSLOT - NOWN, np.int64)])
        sds = np.concatenate([sdeg[os_], np.zeros(NSLOT - NOWN, np.int64)])
        W1_r = np.maximum(W1_r, dds[::P][:NBL])
        W2_r = np.maximum(W2_r, sds[::P][:NBL])
    W1 = int(W1_r.sum()); W2 = int(W2_r.sum())
    W = max(W1, W2)
    W = ((W + CHUNKS - 1) // CHUNKS) * CHUNKS
    W1 = W2 = W
    B1 = np.concatenate([[0], np.cumsum(W1_r)]).astype(np.int64)
    B2 = np.concatenate([[0], np.cumsum(W2_r)]).astype(np.int64)

    # global sigma_d slot of any node
    gd_slot = (np.arange(N) // NOWN) * NSLOT + sd_map

    M1 = np.zeros((NBL, W1), np.float32)
    M2 = np.zeros((NBL, W2), np.float32)
    for r in range(NBL):
        M1[r, B1[r]:B1[r] + W1_r[r]] = 1.0
        M2[r, B2[r]:B2[r] + W2_r[r]] = 1.0

    # bf16-exact distinct codes for block ids 0..391
    BCODE = np.zeros(NCORES * NBL, np.float64)
    for q in range(NCORES * NBL):
        BCODE[q] = q if q < 256 else (256 + 2 * (q - 256) if q < 384
                                      else 512 + 4 * (q - 384))
    BCODE = BCODE.astype(np.float32)

    bf = mybir.dt.np(BF16)
    cores = []
    for c in range(NCORES):
        lo = c * NOWN
        # ---------- dst-grouped layout (L1 + L2 pass 1) ----------
        md = (dst // NOWN) == c
        es, edl = src[md], dst[md] - lo
        slot = sd_map[lo + edl]
        order = np.argsort(slot, kind="stable")
        sslot = slot[order]; es_o = es[order]
        first = np.searchsorted(sslot, np.arange(NSLOT), side="left")
        j = np.arange(len(sslot)) - first[sslot]
        p = sslot % P; r = sslot // P
        col = B1[r] + j
        xs0 = np.zeros((P, W1), np.float32)
        xs1 = np.zeros((P, W1), np.float32)
        kill1 = np.full((P, W1), -300.0, np.float32)
        g1row = np.zeros((P, W1), np.int64)
        m1sel = np.zeros((P, W1, 64), np.float32)
        xs0[p, col] = x[es_o, 0]
        xs1[p, col] = x[es_o, 1]
        kill1[p, col] = 0.0
        gs = gd_slot[es_o]
        g1row[p, col] = gs // 64
        m1sel[p, col, gs % 64] = 1.0

        # ---------- c-scatter one-hots + pointer-gather aux ----------
        gsrow2 = np.zeros((P, W1), np.int64)     # strided pair-table row
        selpar = np.zeros((P, W1), np.float32)   # which of the 2 row values
        gsblkc = np.zeros((P, W1), np.float32)   # bf16-exact code of gs//128
        ohc = np.zeros((P, W1, P), np.float32)   # src-slot%128 one-hot
        gsrow2[p, col] = gs // 2
        selpar[p, col] = gs % 2
        gsblkc[p, col] = BCODE[gs // P]
        ohc[p, col, gs % P] = 1.0

        # own-node features by sigma_d slot
        kk = np.arange(NOWN)
        nodes_d = od_all[c]                   # node at sigma_d rank k
        xn0 = np.zeros((P, NBL), np.float32)
        xn1 = np.zeros((P, NBL), np.float32)
        xn0[kk % P, kk // P] = x[lo + nodes_d, 0]
        xn1[kk % P, kk // P] = x[lo + nodes_d, 1]

        cores.append(dict(
            xs0=xs0, xs1=xs1, kill1=kill1,
            xn0=xn0, xn1=xn1,
            g1idx=_wrap_idx(gsrow2), selpar=selpar, gsblkc=gsblkc,
            ohc=np.ascontiguousarray(ohc.reshape(P, W1 * P).astype(bf)),
        ))
    iotab = np.ascontiguousarray(
        np.broadcast_to(BCODE[None, :], (P, NCORES * NBL)).astype(bf))
    return (cores, W1, B1.tolist(), W1_r.tolist(), M1, iotab)


# ----------------------------------------------------------------------------
# Device program
# ----------------------------------------------------------------------------

def build_program(W1, B1, W1_r):
    nc = bacc.Bacc("TRN2", target_bir_lowering=False, debug=False,
                   num_devices=NCORES)
    dram = lambda name, shape, dt: nc.dram_tensor(name, shape, dt,
                                                  kind="ExternalInput")
    NI1 = P * W1
    # chunk boundaries: 3 sigma_d blocks per chunk
    CB = [B1[min(3 * k, NBL)] for k in range(NBL // 3 + 1)] + [W1]
    # per-core inputs
    xs0_in = dram("xs0", [P, W1], F32)
    xs1_in = dram("xs1", [P, W1], F32)
    kill1_in = dram("kill1", [P, W1], F32)
    xn0_in = dram("xn0", [P, NBL], F32)
    xn1_in = dram("xn1", [P, NBL], F32)
    g1idx_in = dram("g1idx", [P, NI1 // 16], I16)
    selpar_in = dram("selpar", [P, W1], F32)
    gsblkc_in = dram("gsblkc", [P, W1], F32)
    ohc_in = dram("ohc", [P, W1 * P], BF16)
    # replicated inputs
    M1_in = dram("M1", [NBL, W1], F32)
    iotab_in = dram("iotab", [P, NCORES * NBL], BF16)
    w1f_in = dram("w1f", [1, 256], F32)
    as1_in = dram("as1", [1, 256], F32)
    ad1_in = dram("ad1", [1, 256], F32)
    wh_in = dram("wh", [8, 128], F32)
    b1_in = dram("b1", [P, 1], F32)
    w2_in = dram("w2", [P, 128], F32)
    w2t_in = dram("w2t", [P, 128], F32)
    att2_in = dram("att2", [P, 2], F32)
    b2_in = dram("b2", [1, 128], F32)
    ones_in = dram("ones", [1, 128], F32)
    ident_in = dram("ident", [P, 128], F32)
    out_t = nc.dram_tensor("out", [1, 128], F32, kind="ExternalOutput")

    rg = [list(range(NCORES))]

    with tile.TileContext(nc) as tc:
        with (
            tc.tile_pool(name="const", bufs=1) as cp,
            tc.tile_pool(name="nod", bufs=1) as npl,        # node arrays, full life
            tc.tile_pool(name="work", bufs=1) as wp,
            tc.tile_pool(name="gb", bufs=2) as gbp,         # gather chunk bufs
            tc.tile_pool(name="mb", bufs=2) as mbp,         # mask chunk bufs
            tc.tile_pool(name="psA", bufs=2, space="PSUM") as psA,
            tc.tile_pool(name="psB", bufs=2, space="PSUM") as psB,
            tc.tile_pool(name="psP", bufs=1, space="PSUM") as psP,
            tc.tile_pool(name="dr", bufs=1, space="DRAM") as dp,
        ):
            # ---------- constants ----------
            w1f = cp.tile([1, 256], F32); nc.sync.dma_start(w1f[:], w1f_in[:])
            as1 = cp.tile([1, 256], F32); nc.sync.dma_start(as1[:], as1_in[:])
            ad1 = cp.tile([1, 256], F32); nc.sync.dma_start(ad1[:], ad1_in[:])
            ones = cp.tile([1, 128], F32); nc.sync.dma_start(ones[:], ones_in[:])
            ident = cp.tile([P, 128], F32); nc.sync.dma_start(ident[:], ident_in[:])
            wh = cp.tile([8, 128], F32); nc.sync.dma_start(wh[:], wh_in[:])
            b1c = cp.tile([P, 1], F32); nc.sync.dma_start(b1c[:], b1_in[:])
            w2t = cp.tile([P, 128], F32); nc.sync.dma_start(w2t[:], w2t_in[:])
            att2 = cp.tile([P, 2], F32); nc.sync.dma_start(att2[:], att2_in[:])
            b2r = cp.tile([1, 128], F32); nc.sync.dma_start(b2r[:], b2_in[:])
            m1c = cp.tile([NBL, W1], F32); nc.sync.dma_start(m1c[:], M1_in[:])
            iotab = cp.tile([P, NCORES * NBL], BF16)
            nc.sync.dma_start(iotab[:], iotab_in[:])
            # w2 | wc fused rhs for the per-block node matmul
            w2wc = cp.tile([P, 130], F32)
            nc.sync.dma_start(w2wc[:, 0:128], w2_in[:])
            wcps = psA.tile([P, 2], F32, space="PSUM", tag="small")
            nc.tensor.matmul(wcps[:], lhsT=w2t[:], rhs=att2[:], start=True,
                             stop=True)
            nc.scalar.copy(w2wc[:, 128:130], wcps[:])

            # v = [vs(k,h) | vd(k,h)] on one partition then broadcast
            vt = wp.tile([1, 16], F32, tag="vt")
            for (att, off) in ((as1, 0), (ad1, 8)):
                prod = wp.tile([1, 256], F32, tag="vprod")
                nc.vector.tensor_tensor(
                    out=prod[:], in0=w1f[:], in1=att[:],
                    op=mybir.AluOpType.mult)
                nc.vector.tensor_reduce(
                    out=vt[0:1, off:off + 8].rearrange("p (k h) -> p k h", h=4),
                    in_=prod[0:1, :].rearrange("p (k h c) -> p k h c", h=4, c=32),
                    op=mybir.AluOpType.add, axis=mybir.AxisListType.X)
            vps = psA.tile([P, 16], F32, space="PSUM", tag="small")
            nc.tensor.matmul(vps[:], lhsT=ones[:], rhs=vt[:],
                             start=True, stop=True)
            vrep = cp.tile([P, 16], F32)
            nc.scalar.copy(vrep[:], vps[:])

            # ---------- host edge arrays ----------
            l1_cm = tc.tile_pool(name="l1", bufs=1); l1 = l1_cm.__enter__()
            xs0 = l1.tile([P, W1], F32); nc.sync.dma_start(xs0[:], xs0_in[:])
            xs1 = l1.tile([P, W1], F32); nc.sync.dma_start(xs1[:], xs1_in[:])
            kill1 = npl.tile([P, W1], F32)
            nc.sync.dma_start(kill1[:], kill1_in[:])
            xn0 = cp.tile([P, NBL], F32); nc.sync.dma_start(xn0[:], xn0_in[:])
            xn1 = cp.tile([P, NBL], F32); nc.sync.dma_start(xn1[:], xn1_in[:])

            # ---------- L1: adsum per node, transpose, expand ----------
            adsum = wp.tile([P, 4 * NBL], F32, tag="adsum")
            adv = adsum[:].rearrange("p (h r) -> p h r", h=4)
            tmp49 = wp.tile([P, NBL], F32, tag="tmp49")
            for h in range(4):
                nc.vector.tensor_scalar(
                    out=adv[:, h, :], in0=xn0[:], scalar1=vrep[:, 8 + h:9 + h],
                    scalar2=None, op0=mybir.AluOpType.mult)
                nc.vector.tensor_scalar(
                    out=tmp49[:], in0=xn1[:], scalar1=vrep[:, 12 + h:13 + h],
                    scalar2=None, op0=mybir.AluOpType.mult)
                nc.vector.tensor_tensor(
                    out=adv[:, h, :], in0=adv[:, h, :], in1=tmp49[:],
                    op=mybir.AluOpType.add)
            adsT = wp.tile([NBL, 4 * 128], F32, tag="adsT")
            for h in range(4):
                pt = psA.tile([NBL, 128], F32, space="PSUM", tag="tr")
                nc.tensor.transpose(pt[:], adv[:, h, :], ident[:])
                nc.scalar.copy(adsT[:, h * 128:(h + 1) * 128], pt[:])
            # a1 = a1src + expand(adsum) + kill  (h-major sections)
            a1 = l1.tile([P, 4 * W1], F32)
            a1v = a1[:].rearrange("p (h w) -> p h w", h=4)
            HALF1 = [(0, (W1 + 1) // 2), ((W1 + 1) // 2, W1)]
            for h in range(4):
                for (s0, s1) in HALF1:
                    pe = psB.tile([P, 512], F32, space="PSUM", tag="exp")
                    nc.tensor.matmul(pe[:, 0:s1 - s0],
                                     lhsT=adsT[:, h * 128:(h + 1) * 128],
                                     rhs=m1c[:, s0:s1], start=True, stop=True)
                    nc.vector.tensor_tensor(
                        out=a1v[:, h, s0:s1], in0=pe[:, 0:s1 - s0],
                        in1=kill1[:, s0:s1], op=mybir.AluOpType.add)
            tmpw = l1.tile([P, W1], F32)
            for h in range(4):
                nc.vector.tensor_scalar(
                    out=tmpw[:], in0=xs0[:], scalar1=vrep[:, h:h + 1],
                    scalar2=None, op0=mybir.AluOpType.mult)
                nc.vector.tensor_tensor(
                    out=a1v[:, h, :], in0=a1v[:, h, :], in1=tmpw[:],
                    op=mybir.AluOpType.add)
                nc.vector.tensor_scalar(
                    out=tmpw[:], in0=xs1[:], scalar1=vrep[:, 4 + h:5 + h],
                    scalar2=None, op0=mybir.AluOpType.mult)
                nc.vector.tensor_tensor(
                    out=a1v[:, h, :], in0=a1v[:, h, :], in1=tmpw[:],
                    op=mybir.AluOpType.add)

            # ---------- L1: exp, messages, segment sums ----------
            vals = l1.tile([P, 12 * W1], F32)
            vv = vals[:].rearrange("p (v w) -> p v w", v=12)
            nc.scalar.activation(vals[:, 0:4 * W1], a1[:],
                                 mybir.ActivationFunctionType.Exp)
            nc.scalar.activation(a1[:], a1[:],
                                 mybir.ActivationFunctionType.Exp, scale=NEG)
            nc.vector.tensor_tensor(out=vals[:, 0:4 * W1],
                                    in0=vals[:, 0:4 * W1], in1=a1[:],
                                    op=mybir.AluOpType.max)
            nc.vector.tensor_tensor(
                out=vv[:, 4:8, :], in0=vv[:, 0:4, :],
                in1=xs0[:].rearrange("p (o w) -> p o w", o=1)
                    .to_broadcast([P, 4, W1]),
                op=mybir.AluOpType.mult)
            nc.vector.tensor_tensor(
                out=vv[:, 8:12, :], in0=vv[:, 0:4, :],
                in1=xs1[:].rearrange("p (o w) -> p o w", o=1)
                    .to_broadcast([P, 4, W1]),
                op=mybir.AluOpType.mult)
            sums = wp.tile([P, 12 * NBL], F32, tag="sums")
            sv = sums[:].rearrange("p (v r) -> p v r", v=12)
            for r in range(NBL):
                nc.vector.tensor_reduce(
                    out=sv[:, :, r:r + 1],
                    in_=vv[:, :, B1[r]:B1[r] + W1_r[r]],
                    op=mybir.AluOpType.add, axis=mybir.AxisListType.X)

            # ---------- L1 node phase ----------
            nc.vector.tensor_scalar(out=sv[:, 0:4, :], in0=sv[:, 0:4, :],
                                    scalar1=1e-20, scalar2=None,
                                    op0=mybir.AluOpType.max)
            dr1 = wp.tile([P, 4 * NBL], F32, tag="dr1")
            nc.vector.reciprocal(
                out=dr1[:].rearrange("p (h r) -> p h r", h=4), in_=sv[:, 0:4, :])
            snn = wp.tile([P, NBL * 8], F32, tag="snn")
            nc.vector.tensor_tensor(
                out=snn[:].rearrange("p (r k h) -> p k h r", k=2, h=4),
                in0=sv[:, 4:12, :].rearrange("p (k h) r -> p k h r", k=2),
                in1=dr1[:].rearrange("p (o h r) -> p o h r", o=1, h=4)
                    .to_broadcast([P, 2, 4, NBL]),
                op=mybir.AluOpType.mult)
            # per-block fused node pipeline: snn_r -> snt_r -> y_r -> h2n/a2
            h2n = npl.tile([P, NSLOT], F32)
            asown = npl.tile([P, NBL], F32)
            adown = npl.tile([P, NBL], F32)
            for r in range(NBL):
                pt = psA.tile([8, 128], F32, space="PSUM", tag="tr")
                nc.tensor.transpose(pt[:], snn[:, r * 8:(r + 1) * 8], ident[:])
                sntr = gbp.tile([8, 128], F32, tag="sntr")
                nc.scalar.copy(sntr[:], pt[:])
                p1 = psB.tile([P, 512], F32, space="PSUM", tag="exp")
                nc.tensor.matmul(p1[:, 0:128], lhsT=wh[:], rhs=sntr[:],
                                 start=True, stop=True)
                ytr = gbp.tile([P, 128], F32, tag="ytr")
                nc.scalar.activation(ytr[:], p1[:, 0:128],
                                     mybir.ActivationFunctionType.Relu,
                                     bias=b1c[:, 0:1])
                ph = psB.tile([P, 512], F32, space="PSUM", tag="exp")
                nc.tensor.matmul(ph[:, 0:130], lhsT=ytr[:], rhs=w2wc[:],
                                 start=True, stop=True)
                nc.scalar.copy(h2n[:, r * 128:(r + 1) * 128], ph[:, 0:128])
                nc.vector.tensor_copy(out=asown[:, r:r + 1], in_=ph[:, 128:129])
                nc.vector.tensor_copy(out=adown[:, r:r + 1], in_=ph[:, 129:130])

            l1_cm.__exit__(None, None, None)

            # ---------- AllGather 1: a_src2 table ----------
            ag1_in = dp.tile([NSLOT, 1], F32)
            ag1_out = dp.tile([NCORES * NSLOT, 1], F32)
            nc.sync.dma_start(
                ag1_in[:].rearrange("(r p) o -> p (r o)", p=P), asown[:])
            nc.gpsimd.collective_compute(
                "AllGather", mybir.AluOpType.bypass, replica_groups=rg,
                ins=[ag1_in[:]], outs=[ag1_out[:]])
            ag1v = ag1_out[:].rearrange("(q j) o -> q (j o)", j=64)

            # ---------- strided pointer table (2 values / 256B row) ----------
            ptab = dp.tile([NSLOT * NCORES // 2, 64], F32)
            nc.sync.dma_start(
                ptab[:, 0:2],
                ag1_out[:].rearrange("(q t) o -> q (t o)", t=2))

            # ---------- bdexp = expand(a_dst2) + kill ----------
            adT = wp.tile([NBL, 128], F32, tag="adT")
            pt = psA.tile([NBL, 128], F32, space="PSUM", tag="tr")
            nc.tensor.transpose(pt[:], adown[:], ident[:])
            nc.scalar.copy(adT[:], pt[:])
            p1_cm = tc.tile_pool(name="p1", bufs=1); p1l = p1_cm.__enter__()
            g1idx = p1l.tile([P, NI1 // 16], I16)
            nc.sync.dma_start(g1idx[:], g1idx_in[:])
            selpar = p1l.tile([P, W1], F32)
            nc.sync.dma_start(selpar[:], selpar_in[:])
            gsblkc = p1l.tile([P, W1], F32)
            nc.sync.dma_start(gsblkc[:], gsblkc_in[:])
            bdk = p1l.tile([P, W1], F32)     # expand(a_dst2) + kill1
            HALF1 = [(0, (W1 + 1) // 2), ((W1 + 1) // 2, W1)]
            for (s0, s1) in HALF1:
                pe = psB.tile([P, 512], F32, space="PSUM", tag="exp")
                nc.tensor.matmul(pe[:, 0:s1 - s0], lhsT=adT[:],
                                 rhs=m1c[:, s0:s1], start=True, stop=True)
                nc.vector.tensor_tensor(
                    out=bdk[:, s0:s1], in0=pe[:, 0:s1 - s0],
                    in1=kill1[:, s0:s1], op=mybir.AluOpType.add)

            # ---------- chunked: gather a_src2, ex, denom, coef, c-scatter ----
            asg = p1l.tile([P, W1], F32)
            ex1 = p1l.tile([P, W1], F32)
            alpha = p1l.tile([P, W1], F32)
            coef = p1l.tile([P, W1], F32)
            nc.vector.memset(coef[:], 0.0)
            rc = npl.tile([P, NBL], F32)
            ctab = psP.tile([P, NCORES * NBL], F32, space="PSUM", tag="ctab")
            wcnt = 0
            for ci in range(len(CB) - 1):
                c0, c1 = CB[ci], CB[ci + 1]
                cw = c1 - c0
                if cw == 0:
                    continue
                gb = gbp.tile([P, W1 // 8 * 2 + 16], F32, tag="g")
                eng = nc.gpsimd
                eng.add_instruction(mybir.InstDMAGatherAnt(
                    name=nc.get_next_instruction_name(),
                    ins=[*eng.lower_ap_dma(ptab[:, 0:2], for_custom_bir_dma=True),
                         eng.lower_ap(g1idx[:, c0 * 8:c1 * 8]),
                         eng.lower_val_access(eng.to_reg(P * cw))],
                    outs=[eng.lower_ap(
                        gb[:, 0:cw * 2].rearrange("p (w j) -> p w j", j=2))],
                    transpose=False, num_idxs=P * cw, elem_size=2,
                    stride_bytes_256=1, gen_mode=0, single_packet=False,
                    queue_num=0, sbuf_tokens_per_rank=0,
                    sbuf_free_dim_per_rank=0, sbuf_free_dim_pad_per_rank=0,
                    sbuf_byte_offset=0))
                gv = gb[:, 0:cw * 2].rearrange("p (w j) -> p w j", j=2)
                # select 1-of-2:  asg = g0 + (g1-g0)*selpar
                dsel = gbp.tile([P, W1 // 8 + 16], F32, tag="dsel")
                nc.vector.tensor_tensor(out=dsel[:, 0:cw], in0=gv[:, :, 1],
                                        in1=gv[:, :, 0],
                                        op=mybir.AluOpType.subtract)
                nc.vector.tensor_tensor(out=dsel[:, 0:cw], in0=dsel[:, 0:cw],
                                        in1=selpar[:, c0:c1],
                                        op=mybir.AluOpType.mult)
                nc.vector.tensor_tensor(out=asg[:, c0:c1], in0=dsel[:, 0:cw],
                                        in1=gv[:, :, 0],
                                        op=mybir.AluOpType.add)
                nc.vector.tensor_tensor(out=alpha[:, c0:c1], in0=asg[:, c0:c1],
                                        in1=bdk[:, c0:c1],
                                        op=mybir.AluOpType.add)
                nc.scalar.activation(ex1[:, c0:c1], alpha[:, c0:c1],
                                     mybir.ActivationFunctionType.Exp)
                nc.scalar.activation(alpha[:, c0:c1], alpha[:, c0:c1],
                                     mybir.ActivationFunctionType.Exp,
                                     scale=NEG)
                nc.vector.tensor_tensor(out=ex1[:, c0:c1], in0=ex1[:, c0:c1],
                                        in1=alpha[:, c0:c1],
                                        op=mybir.AluOpType.max)
                # blocks fully inside this chunk
                ohb = mbp.tile([P, (W1 // 8 + 16) * P], BF16, tag="ohc")
                nc.sync.dma_start(ohb[:, 0:cw * P],
                                  ohc_in[:, c0 * P:c1 * P])
                for r in range(3 * ci, min(3 * ci + 3, NBL)):
                    b0, b1r = B1[r], B1[r] + W1_r[r]
                    if b1r > b0:
                        nc.vector.tensor_reduce(
                            out=rc[:, r:r + 1],
                            in_=ex1[:, b0:b1r],
                            op=mybir.AluOpType.add, axis=mybir.AxisListType.X)
                    nc.vector.tensor_scalar(
                        out=rc[:, r:r + 1], in0=rc[:, r:r + 1], scalar1=1e-20,
                        scalar2=None, op0=mybir.AluOpType.max)
                    nc.vector.reciprocal(out=rc[:, r:r + 1], in_=rc[:, r:r + 1])
                    if b1r > b0:
                        nc.vector.tensor_scalar(
                            out=coef[:, b0:b1r], in0=ex1[:, b0:b1r],
                            scalar1=rc[:, r:r + 1], scalar2=None,
                            op0=mybir.AluOpType.mult)
                for w in range(c0, c1):
                    spr = gbp.tile([P, NCORES * NBL], BF16, tag="spr")
                    nc.vector.tensor_scalar(
                        out=spr[:], in0=iotab[:],
                        scalar1=gsblkc[:, w:w + 1], scalar2=coef[:, w:w + 1],
                        op0=mybir.AluOpType.is_equal,
                        op1=mybir.AluOpType.mult)
                    nc.tensor.matmul(
                        ctab[:], lhsT=ohb[:, (w - c0) * P:(w - c0 + 1) * P],
                        rhs=spr[:], start=(wcnt == 0), stop=(wcnt == W1 - 1))
                    wcnt += 1
            assert wcnt == W1

            p1_cm.__exit__(None, None, None)

            # ---------- ReduceScatter c-table, transpose own shard ----------
            ctsb = wp.tile([P, NCORES * NBL], F32, tag="ctsb")
            nc.scalar.copy(ctsb[:], ctab[:])
            ctT_d = dp.tile([NCORES * NSLOT, 1], F32)   # flat (B, m)
            ctTv = ctT_d[:].rearrange("(b m) o -> b (m o)", m=P)
            for k in range(4):
                ptk = psA.tile([98, 128], F32, space="PSUM", tag="tr")
                nc.tensor.transpose(ptk[:], ctsb[:, k * 98:(k + 1) * 98],
                                    ident[:])
                tk = wp.tile([98, 128], F32, tag="tk")
                nc.scalar.copy(tk[:], ptk[:])
                nc.sync.dma_start(ctTv[k * 98:(k + 1) * 98, :], tk[:])
            cs_out = dp.tile([NSLOT, 1], F32)
            nc.gpsimd.collective_compute(
                "ReduceScatter", mybir.AluOpType.add, replica_groups=rg,
                ins=[ctT_d[:]], outs=[cs_out[:]])
            shard = wp.tile([NBL, 128], F32, tag="shard")
            nc.sync.dma_start(shard[:],
                              cs_out[:].rearrange("(b m) o -> b (m o)", m=P))
            ptc = psA.tile([P, NBL], F32, space="PSUM", tag="tr")
            nc.tensor.transpose(ptc[:], shard[:], ident[0:NBL, 0:NBL])
            c_d = wp.tile([P, NBL], F32, tag="cd")
            nc.scalar.copy(c_d[:], ptc[:])

            # ---------- final P = sum_n c[n] h2[n]; AllReduce; output ----------
            pps = psP.tile([P, 1], F32, space="PSUM", tag="pfin")
            for r in range(NBL):
                nc.tensor.matmul(pps[:], lhsT=h2n[:, r * 128:(r + 1) * 128],
                                 rhs=c_d[:, r:r + 1],
                                 start=(r == 0), stop=(r == NBL - 1))
            pcol = wp.tile([P, 1], F32, tag="pcol")
            nc.scalar.copy(pcol[:], pps[:])
            ar_in = dp.tile([P, 1], F32)
            ar_out = dp.tile([P, 1], F32)
            nc.sync.dma_start(ar_in[:], pcol[:])
            nc.gpsimd.collective_compute(
                "AllReduce", mybir.AluOpType.add, replica_groups=rg,
                ins=[ar_in[:]], outs=[ar_out[:]])
            prow = wp.tile([1, 128], F32, tag="prow")
            nc.sync.dma_start(prow[:], ar_out[:].rearrange("(o f) j -> o (f j)", o=1))
            res = wp.tile([1, 128], F32, tag="res")
            nc.vector.tensor_scalar(out=res[:], in0=prow[:], scalar1=1.0 / N,
                                    scalar2=None, op0=mybir.AluOpType.mult)
            nc.vector.tensor_tensor(out=res[:], in0=res[:], in1=b2r[:],
                                    op=mybir.AluOpType.add)
            nc.sync.dma_start(out_t[:], res[:])

    nc.compile()
    return nc


# ----------------------------------------------------------------------------
# Entry point
# ----------------------------------------------------------------------------

def kernel(x, edge_index, W1, att_src1, att_dst1, b1, W2, att_src2, att_dst2,
           b2, _trace=False):
    x = np.asarray(x, np.float32)
    edge_index = np.asarray(edge_index, np.int64)
    key = "prog"
    if key not in _CACHE:
        cores, w1, B1, W1r, M1, iotab = host_prep(x, edge_index)
        nc = build_program(w1, B1, W1r)
        _CACHE[key] = (nc, cores, M1, iotab)
    nc, cores, M1, iotab = _CACHE[key]

    shared = dict(
        M1=M1, iotab=iotab,
        w1f=np.asarray(W1, np.float32).reshape(1, 256),
        as1=np.tile(np.asarray(att_src1, np.float32).reshape(128), 2)
            .reshape(1, 256),
        ad1=np.tile(np.asarray(att_dst1, np.float32).reshape(128), 2)
            .reshape(1, 256),
        b1=np.asarray(b1, np.float32).reshape(P, 1),
        w2=np.ascontiguousarray(np.asarray(W2, np.float32)),
        w2t=np.ascontiguousarray(np.asarray(W2, np.float32).T),
        att2=np.ascontiguousarray(np.stack(
            [np.asarray(att_src2, np.float32).reshape(128),
             np.asarray(att_dst2, np.float32).reshape(128)], axis=1)),
        b2=np.asarray(b2, np.float32).reshape(1, 128),
        ones=np.ones((1, 128), np.float32),
        ident=np.eye(128, dtype=np.float32),
    )
    # W-hat: Wh[k*4+h, h*32+c] = W1[k, h*32+c]
    W1a = np.asarray(W1, np.float32)
    wh = np.zeros((8, 128), np.float32)
    for h in range(4):
        for k in range(2):
            wh[4 * k + h, h * 32:(h + 1) * 32] = W1a[k, h * 32:(h + 1) * 32]
    shared["wh"] = wh

    in_maps = []
    for c in range(NCORES):
        m = dict(shared)
        m.update(cores[c])
        in_maps.append(m)
    res = run_bass_kernel_spmd(nc, in_maps, core_ids=list(range(NCORES)),
                               trace=_trace)
    out = res.results[0]["out"].reshape(128).astype(np.float32)
    kernel.last_exec_ns = res.exec_time_ns
    return out


# revision 16
# speedup vs baseline: 1.0146x; 1.0146x over previous
"""GAT encoder (2-layer, PyG-style) on 8 Trainium2 NeuronCores.

v2 architecture — "lane layout" edge processing, no per-tile loops:
  - Nodes sharded by range across 8 cores (6250 own/core). Per core, TWO
    node->slot bijections: sigma_d (sorted by in-degree) for all node arrays
    and the dst-grouped edge layout; sigma_s (sorted by out-degree) for the
    src-grouped edge layout of layer-2 pass 2.
  - Edge arrays are [128, W]: edge (s->d) sits at partition sd(d)%128,
    in the column range of block sd(d)//128 (degree-sorted packing makes
    W ~= E/128 with ~98% density). Per-dst-node values broadcast via ONE
    wide matmul (block one-hot), per-dst segment sums via 49 free-dim
    reduces. Zero per-tile one-hot/transpose machinery.
  - Per-edge values keyed by the *other* endpoint are fetched with
    dma_gather (SWDGE, 256B rows, one instruction per chunk) from
    AllGathered tables, then selected with host-baked bf16 one-hot masks:
    gpsimd mult + DVE inner reduce.
  - Layer 2: pass 1 (dst layout) computes softmax denominators; AllGather
    (a_dst2, 1/denom) pairs; pass 2 (src layout) computes per-edge coef and
    reduces c[s] = sum coef by src. Final P = sum_n c[n] h2[n], AllReduce.
"""

import sys
import numpy as np

sys.path.insert(0, "/opt/trn_rl_repo")

import concourse.bass as bass
import concourse.bacc as bacc
import concourse.mybir as mybir
import concourse.tile as tile
from concourse.bass_utils import run_bass_kernel_spmd

P = 128
NCORES = 8
N = 50000
NOWN = N // NCORES          # 6250
NBL = 49                    # blocks per core (49*128 = 6272 slots)
NSLOT = NBL * P
NEG = 0.2
CHUNKS = 16

F32 = mybir.dt.float32
BF16 = mybir.dt.bfloat16
I16 = mybir.dt.int16

_CACHE = {}


# ----------------------------------------------------------------------------
# Host-side prep: pure index/permutation work (gathers of x, one-hot masks).
# ----------------------------------------------------------------------------

def _wrap_idx(idxmat):
    """[128, W] row-index matrix -> dma_gather idx tile [128, 128*W//16] i16.
    Logical idx order i = w*128 + p; wrapped in 16 partitions, replicated
    across the 8 Q7 cores."""
    ni = idxmat.size
    lst = idxmat.T.reshape(ni)                      # lst[i] = idxmat[i%128, i//128]
    sb = lst.reshape(ni // 16, 16).T                # [16, ni/16]
    return np.ascontiguousarray(np.tile(sb, (8, 1)).astype(np.int16))


def host_prep(x, edge_index):
    src = np.concatenate([edge_index[0], np.arange(N)]).astype(np.int64)
    dst = np.concatenate([edge_index[1], np.arange(N)]).astype(np.int64)

    # --- per-core slot assignments + global block widths ---
    sd_map = np.zeros(N, np.int64)    # node -> sigma_d slot (local)
    ss_map = np.zeros(N, np.int64)    # node -> sigma_s slot (local)
    od_all, os_all = [], []
    W1_r = np.zeros(NBL, np.int64)
    W2_r = np.zeros(NBL, np.int64)
    for c in range(NCORES):
        lo = c * NOWN
        dd = np.bincount(dst[(dst // NOWN) == c] - lo, minlength=NOWN)
        sdeg = np.bincount(src[(src // NOWN) == c] - lo, minlength=NOWN)
        od = np.argsort(-dd, kind="stable")
        os_ = np.argsort(-sdeg, kind="stable")
        rd = np.empty(NOWN, np.int64); rd[od] = np.arange(NOWN)
        rs = np.empty(NOWN, np.int64); rs[os_] = np.arange(NOWN)
        sd_map[lo:lo + NOWN] = rd
        ss_map[lo:lo + NOWN] = rs
        od_all.append(od); os_all.append(os_)
        dds = np.concatenate([dd[od], np.zeros(NSLOT - NOWN, np.int64)])
        sds = np.concatenate([sdeg[os_], np.zeros(NSLOT - NOWN, np.int64)])
        W1_r = np.maximum(W1_r, dds[::P][:NBL])
        W2_r = np.maximum(W2_r, sds[::P][:NBL])
    W1 = int(W1_r.sum()); W2 = int(W2_r.sum())
    W = max(W1, W2)
    W = ((W + CHUNKS - 1) // CHUNKS) * CHUNKS
    W1 = W2 = W
    B1 = np.concatenate([[0], np.cumsum(W1_r)]).astype(np.int64)
    B2 = np.concatenate([[0], np.cumsum(W2_r)]).astype(np.int64)

    # global sigma_d slot of any node
    gd_slot = (np.arange(N) // NOWN) * NSLOT + sd_map

    M1 = np.zeros((NBL, W1), np.float32)
    M2 = np.zeros((NBL, W2), np.float32)
    for r in range(NBL):
        M1[r, B1[r]:B1[r] + W1_r[r]] = 1.0
        M2[r, B2[r]:B2[r] + W2_r[r]] = 1.0

    # bf16-exact distinct codes for block ids 0..391
    BCODE = np.zeros(NCORES * NBL, np.float64)
    for q in range(NCORES * NBL):
        BCODE[q] = q if q < 256 else (256 + 2 * (q - 256) if q < 384
                                      else 512 + 4 * (q - 384))
    BCODE = BCODE.astype(np.float32)

    bf = mybir.dt.np(BF16)
    cores = []
    for c in range(NCORES):
        lo = c * NOWN
        # ---------- dst-grouped layout (L1 + L2 pass 1) ----------
        md = (dst // NOWN) == c
        es, edl = src[md], dst[md] - lo
        slot = sd_map[lo + edl]
        order = np.argsort(slot, kind="stable")
        sslot = slot[order]; es_o = es[order]
        first = np.searchsorted(sslot, np.arange(NSLOT), side="left")
        j = np.arange(len(sslot)) - first[sslot]
        p = sslot % P; r = sslot // P
        col = B1[r] + j
        xs0 = np.zeros((P, W1), np.float32)
        xs1 = np.zeros((P, W1), np.float32)
        kill1 = np.full((P, W1), -300.0, np.float32)
        g1row = np.zeros((P, W1), np.int64)
        m1sel = np.zeros((P, W1, 64), np.float32)
        xs0[p, col] = x[es_o, 0]
        xs1[p, col] = x[es_o, 1]
        kill1[p, col] = 0.0
        gs = gd_slot[es_o]
        g1row[p, col] = gs // 64
        m1sel[p, col, gs % 64] = 1.0

        # ---------- c-scatter one-hots + pointer-gather aux ----------
        gsrow2 = np.zeros((P, W1), np.int64)     # strided pair-table row
        selpar = np.zeros((P, W1), np.float32)   # which of the 2 row values
        ohc = np.zeros((P, W1, P), np.float32)   # src-slot%128 one-hot
        sprh = np.zeros((P, W1, NCORES * NBL), bf)  # src block one-hot
        gsrow2[p, col] = gs // 2
        selpar[p, col] = gs % 2
        ohc[p, col, gs % P] = 1.0
        sprh[p, col, gs // P] = 1.0

        # own-node features by sigma_d slot
        kk = np.arange(NOWN)
        nodes_d = od_all[c]                   # node at sigma_d rank k
        xn0 = np.zeros((P, NBL), np.float32)
        xn1 = np.zeros((P, NBL), np.float32)
        xn0[kk % P, kk // P] = x[lo + nodes_d, 0]
        xn1[kk % P, kk // P] = x[lo + nodes_d, 1]

        cores.append(dict(
            xs0=xs0, xs1=xs1, kill1=kill1,
            xn0=xn0, xn1=xn1,
            g1idx=_wrap_idx(gsrow2), selpar=selpar,
            ohc=np.ascontiguousarray(ohc.reshape(P, W1 * P).astype(bf)),
            sprh=np.ascontiguousarray(sprh.reshape(P, W1 * NCORES * NBL)),
        ))
    return (cores, W1, B1.tolist(), W1_r.tolist(), M1)


# ----------------------------------------------------------------------------
# Device program
# ----------------------------------------------------------------------------

def build_program(W1, B1, W1_r):
    nc = bacc.Bacc("TRN2", target_bir_lowering=False, debug=False,
                   num_devices=NCORES)
    dram = lambda name, shape, dt: nc.dram_tensor(name, shape, dt,
                                                  kind="ExternalInput")
    NI1 = P * W1
    MAXW = max(W1_r)
    # chunk boundaries: 3 sigma_d blocks per chunk
    CB = [B1[min(3 * k, NBL)] for k in range(NBL // 3 + 1)] + [W1]
    # per-core inputs
    xs0_in = dram("xs0", [P, W1], F32)
    xs1_in = dram("xs1", [P, W1], F32)
    kill1_in = dram("kill1", [P, W1], F32)
    xn0_in = dram("xn0", [P, NBL], F32)
    xn1_in = dram("xn1", [P, NBL], F32)
    g1idx_in = dram("g1idx", [P, NI1 // 16], I16)
    selpar_in = dram("selpar", [P, W1], F32)
    ohc_in = dram("ohc", [P, W1 * P], BF16)
    sprh_in = dram("sprh", [P, W1 * NCORES * NBL], BF16)
    # replicated inputs
    M1_in = dram("M1", [NBL, W1], F32)
    w1f_in = dram("w1f", [1, 256], F32)
    as1_in = dram("as1", [1, 256], F32)
    ad1_in = dram("ad1", [1, 256], F32)
    wh_in = dram("wh", [8, 128], F32)
    b1_in = dram("b1", [P, 1], F32)
    w2_in = dram("w2", [P, 128], F32)
    w2t_in = dram("w2t", [P, 128], F32)
    att2_in = dram("att2", [P, 2], F32)
    b2_in = dram("b2", [1, 128], F32)
    ones_in = dram("ones", [1, 128], F32)
    ident_in = dram("ident", [P, 128], F32)
    out_t = nc.dram_tensor("out", [1, 128], F32, kind="ExternalOutput")

    rg = [list(range(NCORES))]

    with tile.TileContext(nc) as tc:
        with (
            tc.tile_pool(name="const", bufs=1) as cp,
            tc.tile_pool(name="nod", bufs=1) as npl,        # node arrays, full life
            tc.tile_pool(name="work", bufs=1) as wp,
            tc.tile_pool(name="gb", bufs=2) as gbp,         # gather chunk bufs
            tc.tile_pool(name="mb", bufs=2) as mbp,
            tc.tile_pool(name="sprp", bufs=2) as sprp,         # mask chunk bufs
            tc.tile_pool(name="psA", bufs=2, space="PSUM") as psA,
            tc.tile_pool(name="psB", bufs=2, space="PSUM") as psB,
            tc.tile_pool(name="psP", bufs=1, space="PSUM") as psP,
            tc.tile_pool(name="dr", bufs=1, space="DRAM") as dp,
        ):
            # ---------- constants ----------
            w1f = cp.tile([1, 256], F32); nc.sync.dma_start(w1f[:], w1f_in[:])
            as1 = cp.tile([1, 256], F32); nc.sync.dma_start(as1[:], as1_in[:])
            ad1 = cp.tile([1, 256], F32); nc.sync.dma_start(ad1[:], ad1_in[:])
            ones = cp.tile([1, 128], F32); nc.sync.dma_start(ones[:], ones_in[:])
            ident = cp.tile([P, 128], F32); nc.sync.dma_start(ident[:], ident_in[:])
            wh = cp.tile([8, 128], F32); nc.sync.dma_start(wh[:], wh_in[:])
            b1c = cp.tile([P, 1], F32); nc.sync.dma_start(b1c[:], b1_in[:])
            w2t = cp.tile([P, 128], F32); nc.sync.dma_start(w2t[:], w2t_in[:])
            att2 = cp.tile([P, 2], F32); nc.sync.dma_start(att2[:], att2_in[:])
            b2r = cp.tile([1, 128], F32); nc.sync.dma_start(b2r[:], b2_in[:])
            m1c = cp.tile([NBL, W1], F32); nc.sync.dma_start(m1c[:], M1_in[:])
            # w2 | wc fused rhs for the per-block node matmul
            w2wc = cp.tile([P, 130], F32)
            nc.sync.dma_start(w2wc[:, 0:128], w2_in[:])
            wcps = psA.tile([P, 2], F32, space="PSUM", tag="small")
            nc.tensor.matmul(wcps[:], lhsT=w2t[:], rhs=att2[:], start=True,
                             stop=True)
            nc.scalar.copy(w2wc[:, 128:130], wcps[:])

            # v = [vs(k,h) | vd(k,h)] on one partition then broadcast
            vt = wp.tile([1, 16], F32, tag="vt")
            for (att, off) in ((as1, 0), (ad1, 8)):
                prod = wp.tile([1, 256], F32, tag="vprod")
                nc.vector.tensor_tensor(
                    out=prod[:], in0=w1f[:], in1=att[:],
                    op=mybir.AluOpType.mult)
                nc.vector.tensor_reduce(
                    out=vt[0:1, off:off + 8].rearrange("p (k h) -> p k h", h=4),
                    in_=prod[0:1, :].rearrange("p (k h c) -> p k h c", h=4, c=32),
                    op=mybir.AluOpType.add, axis=mybir.AxisListType.X)
            vps = psA.tile([P, 16], F32, space="PSUM", tag="small")
            nc.tensor.matmul(vps[:], lhsT=ones[:], rhs=vt[:],
                             start=True, stop=True)
            vrep = cp.tile([P, 16], F32)
            nc.scalar.copy(vrep[:], vps[:])

            # ---------- host edge arrays ----------
            l1_cm = tc.tile_pool(name="l1", bufs=1); l1 = l1_cm.__enter__()
            xs0 = l1.tile([P, W1], F32); nc.sync.dma_start(xs0[:], xs0_in[:])
            xs1 = l1.tile([P, W1], F32); nc.sync.dma_start(xs1[:], xs1_in[:])
            kill1 = npl.tile([P, W1], F32)
            nc.sync.dma_start(kill1[:], kill1_in[:])
            xn0 = cp.tile([P, NBL], F32); nc.sync.dma_start(xn0[:], xn0_in[:])
            xn1 = cp.tile([P, NBL], F32); nc.sync.dma_start(xn1[:], xn1_in[:])

            # ---------- L1: adsum per node, transpose, expand ----------
            adsum = wp.tile([P, 4 * NBL], F32, tag="adsum")
            adv = adsum[:].rearrange("p (h r) -> p h r", h=4)
            tmp49 = wp.tile([P, NBL], F32, tag="tmp49")
            for h in range(4):
                nc.vector.tensor_scalar(
                    out=adv[:, h, :], in0=xn0[:], scalar1=vrep[:, 8 + h:9 + h],
                    scalar2=None, op0=mybir.AluOpType.mult)
                nc.vector.tensor_scalar(
                    out=tmp49[:], in0=xn1[:], scalar1=vrep[:, 12 + h:13 + h],
                    scalar2=None, op0=mybir.AluOpType.mult)
                nc.vector.tensor_tensor(
                    out=adv[:, h, :], in0=adv[:, h, :], in1=tmp49[:],
                    op=mybir.AluOpType.add)
            adsT = wp.tile([NBL, 4 * 128], F32, tag="adsT")
            for h in range(4):
                pt = psA.tile([NBL, 128], F32, space="PSUM", tag="tr")
                nc.tensor.transpose(pt[:], adv[:, h, :], ident[:])
                nc.scalar.copy(adsT[:, h * 128:(h + 1) * 128], pt[:])
            # a1 = a1src + expand(adsum) + kill  (h-major sections)
            a1 = l1.tile([P, 4 * W1], F32)
            a1v = a1[:].rearrange("p (h w) -> p h w", h=4)
            HALF1 = [(0, (W1 + 1) // 2), ((W1 + 1) // 2, W1)]
            for h in range(4):
                for (s0, s1) in HALF1:
                    pe = psB.tile([P, 512], F32, space="PSUM", tag="exp")
                    nc.tensor.matmul(pe[:, 0:s1 - s0],
                                     lhsT=adsT[:, h * 128:(h + 1) * 128],
                                     rhs=m1c[:, s0:s1], start=True, stop=True)
                    nc.vector.tensor_tensor(
                        out=a1v[:, h, s0:s1], in0=pe[:, 0:s1 - s0],
                        in1=kill1[:, s0:s1], op=mybir.AluOpType.add)
            tmpw = l1.tile([P, W1], F32)
            for h in range(4):
                nc.vector.tensor_scalar(
                    out=tmpw[:], in0=xs0[:], scalar1=vrep[:, h:h + 1],
                    scalar2=None, op0=mybir.AluOpType.mult)
                nc.vector.tensor_tensor(
                    out=a1v[:, h, :], in0=a1v[:, h, :], in1=tmpw[:],
                    op=mybir.AluOpType.add)
                nc.vector.tensor_scalar(
                    out=tmpw[:], in0=xs1[:], scalar1=vrep[:, 4 + h:5 + h],
                    scalar2=None, op0=mybir.AluOpType.mult)
                nc.vector.tensor_tensor(
                    out=a1v[:, h, :], in0=a1v[:, h, :], in1=tmpw[:],
                    op=mybir.AluOpType.add)

            # ---------- L1: exp, messages, segment sums ----------
            vals = l1.tile([P, 12 * W1], F32)
            vv = vals[:].rearrange("p (v w) -> p v w", v=12)
            nc.scalar.activation(vals[:, 0:4 * W1], a1[:],
                                 mybir.ActivationFunctionType.Exp)
            nc.scalar.activation(a1[:], a1[:],
                                 mybir.ActivationFunctionType.Exp, scale=NEG)
            nc.vector.tensor_tensor(out=vals[:, 0:4 * W1],
                                    in0=vals[:, 0:4 * W1], in1=a1[:],
                                    op=mybir.AluOpType.max)
            nc.vector.tensor_tensor(
                out=vv[:, 4:8, :], in0=vv[:, 0:4, :],
                in1=xs0[:].rearrange("p (o w) -> p o w", o=1)
                    .to_broadcast([P, 4, W1]),
                op=mybir.AluOpType.mult)
            nc.vector.tensor_tensor(
                out=vv[:, 8:12, :], in0=vv[:, 0:4, :],
                in1=xs1[:].rearrange("p (o w) -> p o w", o=1)
                    .to_broadcast([P, 4, W1]),
                op=mybir.AluOpType.mult)
            sums = wp.tile([P, 12 * NBL], F32, tag="sums")
            sv = sums[:].rearrange("p (v r) -> p v r", v=12)
            for r in range(NBL):
                nc.vector.tensor_reduce(
                    out=sv[:, :, r:r + 1],
                    in_=vv[:, :, B1[r]:B1[r] + W1_r[r]],
                    op=mybir.AluOpType.add, axis=mybir.AxisListType.X)

            # ---------- L1 node phase ----------
            nc.vector.tensor_scalar(out=sv[:, 0:4, :], in0=sv[:, 0:4, :],
                                    scalar1=1e-20, scalar2=None,
                                    op0=mybir.AluOpType.max)
            dr1 = wp.tile([P, 4 * NBL], F32, tag="dr1")
            nc.vector.reciprocal(
                out=dr1[:].rearrange("p (h r) -> p h r", h=4), in_=sv[:, 0:4, :])
            snn = wp.tile([P, NBL * 8], F32, tag="snn")
            nc.vector.tensor_tensor(
                out=snn[:].rearrange("p (r k h) -> p k h r", k=2, h=4),
                in0=sv[:, 4:12, :].rearrange("p (k h) r -> p k h r", k=2),
                in1=dr1[:].rearrange("p (o h r) -> p o h r", o=1, h=4)
                    .to_broadcast([P, 2, 4, NBL]),
                op=mybir.AluOpType.mult)
            # per-block fused node pipeline: snn_r -> snt_r -> y_r -> h2n/a2
            h2n = npl.tile([P, NSLOT], F32)
            asown = npl.tile([P, NBL], F32)
            adown = npl.tile([P, NBL], F32)
            for r in range(NBL):
                pt = psA.tile([8, 128], F32, space="PSUM", tag="tr")
                nc.tensor.transpose(pt[:], snn[:, r * 8:(r + 1) * 8], ident[:])
                sntr = gbp.tile([8, 128], F32, tag="sntr")
                nc.scalar.copy(sntr[:], pt[:])
                p1 = psB.tile([P, 512], F32, space="PSUM", tag="exp")
                nc.tensor.matmul(p1[:, 0:128], lhsT=wh[:], rhs=sntr[:],
                                 start=True, stop=True)
                ytr = gbp.tile([P, 128], F32, tag="ytr")
                nc.scalar.activation(ytr[:], p1[:, 0:128],
                                     mybir.ActivationFunctionType.Relu,
                                     bias=b1c[:, 0:1])
                ph = psB.tile([P, 512], F32, space="PSUM", tag="exp")
                nc.tensor.matmul(ph[:, 0:130], lhsT=ytr[:], rhs=w2wc[:],
                                 start=True, stop=True)
                nc.scalar.copy(h2n[:, r * 128:(r + 1) * 128], ph[:, 0:128])
                nc.vector.tensor_copy(out=asown[:, r:r + 1], in_=ph[:, 128:129])
                nc.vector.tensor_copy(out=adown[:, r:r + 1], in_=ph[:, 129:130])

            l1_cm.__exit__(None, None, None)

            # ---------- AllGather 1: a_src2 table ----------
            ag1_in = dp.tile([NSLOT, 1], F32)
            ag1_out = dp.tile([NCORES * NSLOT, 1], F32)
            nc.sync.dma_start(
                ag1_in[:].rearrange("(r p) o -> p (r o)", p=P), asown[:])
            nc.gpsimd.collective_compute(
                "AllGather", mybir.AluOpType.bypass, replica_groups=rg,
                ins=[ag1_in[:]], outs=[ag1_out[:]])
            ag1v = ag1_out[:].rearrange("(q j) o -> q (j o)", j=64)

            # ---------- strided pointer table (2 values / 256B row) ----------
            ptab = dp.tile([NSLOT * NCORES // 2, 64], F32)
            nc.sync.dma_start(
                ptab[:, 0:2],
                ag1_out[:].rearrange("(q t) o -> q (t o)", t=2))

            # ---------- bdexp = expand(a_dst2) + kill ----------
            adT = wp.tile([NBL, 128], F32, tag="adT")
            pt = psA.tile([NBL, 128], F32, space="PSUM", tag="tr")
            nc.tensor.transpose(pt[:], adown[:], ident[:])
            nc.scalar.copy(adT[:], pt[:])
            p1_cm = tc.tile_pool(name="p1", bufs=1); p1l = p1_cm.__enter__()
            g1idx = p1l.tile([P, NI1 // 16], I16)
            nc.sync.dma_start(g1idx[:], g1idx_in[:])
            selpar = p1l.tile([P, W1], F32)
            nc.sync.dma_start(selpar[:], selpar_in[:])
            bdk = p1l.tile([P, W1], F32)     # expand(a_dst2) + kill1
            HALF1 = [(0, (W1 + 1) // 2), ((W1 + 1) // 2, W1)]
            for (s0, s1) in HALF1:
                pe = psB.tile([P, 512], F32, space="PSUM", tag="exp")
                nc.tensor.matmul(pe[:, 0:s1 - s0], lhsT=adT[:],
                                 rhs=m1c[:, s0:s1], start=True, stop=True)
                nc.vector.tensor_tensor(
                    out=bdk[:, s0:s1], in0=pe[:, 0:s1 - s0],
                    in1=kill1[:, s0:s1], op=mybir.AluOpType.add)

            # ---------- chunked: gather a_src2, ex, denom, coef, c-scatter ----
            asg = p1l.tile([P, W1], F32)
            ex1 = p1l.tile([P, W1], F32)
            alpha = p1l.tile([P, W1], F32)
            coef = p1l.tile([P, W1], BF16)
            nc.vector.memset(coef[:], 0.0)
            rc = npl.tile([P, NBL], F32)
            ctab = psP.tile([P, NCORES * NBL], F32, space="PSUM", tag="ctab")
            wcnt = 0
            for ci in range(len(CB) - 1):
                c0, c1 = CB[ci], CB[ci + 1]
                cw = c1 - c0
                if cw == 0:
                    continue
                gb = gbp.tile([P, W1 // 8 * 2 + 16], F32, tag="g")
                eng = nc.gpsimd
                eng.add_instruction(mybir.InstDMAGatherAnt(
                    name=nc.get_next_instruction_name(),
                    ins=[*eng.lower_ap_dma(ptab[:, 0:2], for_custom_bir_dma=True),
                         eng.lower_ap(g1idx[:, c0 * 8:c1 * 8]),
                         eng.lower_val_access(eng.to_reg(P * cw))],
                    outs=[eng.lower_ap(
                        gb[:, 0:cw * 2].rearrange("p (w j) -> p w j", j=2))],
                    transpose=False, num_idxs=P * cw, elem_size=2,
                    stride_bytes_256=1, gen_mode=0, single_packet=False,
                    queue_num=0, sbuf_tokens_per_rank=0,
                    sbuf_free_dim_per_rank=0, sbuf_free_dim_pad_per_rank=0,
                    sbuf_byte_offset=0))
                gv = gb[:, 0:cw * 2].rearrange("p (w j) -> p w j", j=2)
                # select 1-of-2:  asg = g0 + (g1-g0)*selpar
                dsel = gbp.tile([P, W1 // 8 + 16], F32, tag="dsel")
                nc.vector.tensor_tensor(out=dsel[:, 0:cw], in0=gv[:, :, 1],
                                        in1=gv[:, :, 0],
                                        op=mybir.AluOpType.subtract)
                nc.vector.tensor_tensor(out=dsel[:, 0:cw], in0=dsel[:, 0:cw],
                                        in1=selpar[:, c0:c1],
                                        op=mybir.AluOpType.mult)
                nc.vector.tensor_tensor(out=asg[:, c0:c1], in0=dsel[:, 0:cw],
                                        in1=gv[:, :, 0],
                                        op=mybir.AluOpType.add)
                nc.vector.tensor_tensor(out=alpha[:, c0:c1], in0=asg[:, c0:c1],
                                        in1=bdk[:, c0:c1],
                                        op=mybir.AluOpType.add)
                nc.scalar.activation(ex1[:, c0:c1], alpha[:, c0:c1],
                                     mybir.ActivationFunctionType.Exp)
                nc.scalar.activation(alpha[:, c0:c1], alpha[:, c0:c1],
                                     mybir.ActivationFunctionType.Exp,
                                     scale=NEG)
                nc.vector.tensor_tensor(out=ex1[:, c0:c1], in0=ex1[:, c0:c1],
                                        in1=alpha[:, c0:c1],
                                        op=mybir.AluOpType.max)
                # blocks fully inside this chunk
                for r in range(3 * ci, min(3 * ci + 3, NBL)):
                    b0, b1r = B1[r], B1[r] + W1_r[r]
                    cw_r = b1r - b0
                    if cw_r == 0:
                        continue
                    nc.vector.tensor_reduce(
                        out=rc[:, r:r + 1],
                        in_=ex1[:, b0:b1r],
                        op=mybir.AluOpType.add, axis=mybir.AxisListType.X)
                    nc.vector.tensor_scalar(
                        out=rc[:, r:r + 1], in0=rc[:, r:r + 1], scalar1=1e-20,
                        scalar2=None, op0=mybir.AluOpType.max)
                    nc.vector.reciprocal(out=rc[:, r:r + 1], in_=rc[:, r:r + 1])
                    nc.vector.tensor_scalar(
                        out=coef[:, b0:b1r], in0=ex1[:, b0:b1r],
                        scalar1=rc[:, r:r + 1], scalar2=None,
                        op0=mybir.AluOpType.mult)
                    ohb = mbp.tile([P, MAXW * P], BF16, tag="ohc")
                    nc.sync.dma_start(ohb[:, 0:cw_r * P],
                                      ohc_in[:, b0 * P:b1r * P])
                    nc.vector.tensor_tensor(
                        out=ohb[:, 0:cw_r * P].rearrange(
                            "p (w m) -> p w m", m=P),
                        in0=ohb[:, 0:cw_r * P].rearrange(
                            "p (w m) -> p w m", m=P),
                        in1=coef[:, b0:b1r].rearrange(
                            "p (w o) -> p w o", o=1).to_broadcast([P, cw_r, P]),
                        op=mybir.AluOpType.mult)
                    sprb = sprp.tile([P, MAXW * NCORES * NBL], BF16, tag="sprb")
                    nc.sync.dma_start(
                        sprb[:, 0:cw_r * NCORES * NBL],
                        sprh_in[:, b0 * NCORES * NBL:b1r * NCORES * NBL])
                    for j in range(cw_r):
                        nc.tensor.matmul(
                            ctab[:], lhsT=ohb[:, j * P:(j + 1) * P],
                            rhs=sprb[:, j * NCORES * NBL:(j + 1) * NCORES * NBL],
                            start=(wcnt == 0), stop=(wcnt == B1[NBL] - 1))
                        wcnt += 1
            assert wcnt == B1[NBL]

            p1_cm.__exit__(None, None, None)

            # ---------- ReduceScatter c-table, transpose own shard ----------
            ctsb = wp.tile([P, NCORES * NBL], F32, tag="ctsb")
            nc.scalar.copy(ctsb[:], ctab[:])
            ctT_d = dp.tile([NCORES * NSLOT, 1], F32)   # flat (B, m)
            ctTv = ctT_d[:].rearrange("(b m) o -> b (m o)", m=P)
            for k in range(4):
                ptk = psA.tile([98, 128], F32, space="PSUM", tag="tr")
                nc.tensor.transpose(ptk[:], ctsb[:, k * 98:(k + 1) * 98],
                                    ident[:])
                tk = wp.tile([98, 128], F32, tag="tk")
                nc.scalar.copy(tk[:], ptk[:])
                nc.sync.dma_start(ctTv[k * 98:(k + 1) * 98, :], tk[:])
            cs_out = dp.tile([NSLOT, 1], F32)
            nc.gpsimd.collective_compute(
                "ReduceScatter", mybir.AluOpType.add, replica_groups=rg,
                ins=[ctT_d[:]], outs=[cs_out[:]])
            shard = wp.tile([NBL, 128], F32, tag="shard")
            nc.sync.dma_start(shard[:],
                              cs_out[:].rearrange("(b m) o -> b (m o)", m=P))
            ptc = psA.tile([P, NBL], F32, space="PSUM", tag="tr")
            nc.tensor.transpose(ptc[:], shard[:], ident[0:NBL, 0:NBL])
            c_d = wp.tile([P, NBL], F32, tag="cd")
            nc.scalar.copy(c_d[:], ptc[:])

            # ---------- final P = sum_n c[n] h2[n]; AllReduce; output ----------
            pps = psP.tile([P, 1], F32, space="PSUM", tag="pfin")
            for r in range(NBL):
                nc.tensor.matmul(pps[:], lhsT=h2n[:, r * 128:(r + 1) * 128],
                                 rhs=c_d[:, r:r + 1],
                                 start=(r == 0), stop=(r == NBL - 1))
            pcol = wp.tile([P, 1], F32, tag="pcol")
            nc.scalar.copy(pcol[:], pps[:])
            ar_in = dp.tile([P, 1], F32)
            ar_out = dp.tile([P, 1], F32)
            nc.sync.dma_start(ar_in[:], pcol[:])
            nc.gpsimd.collective_compute(
                "AllReduce", mybir.AluOpType.add, replica_groups=rg,
                ins=[ar_in[:]], outs=[ar_out[:]])
            prow = wp.tile([1, 128], F32, tag="prow")
            nc.sync.dma_start(prow[:], ar_out[:].rearrange("(o f) j -> o (f j)", o=1))
            res = wp.tile([1, 128], F32, tag="res")
            nc.vector.tensor_scalar(out=res[:], in0=prow[:], scalar1=1.0 / N,
                                    scalar2=None, op0=mybir.AluOpType.mult)
            nc.vector.tensor_tensor(out=res[:], in0=res[:], in1=b2r[:],
                                    op=mybir.AluOpType.add)
            nc.sync.dma_start(out_t[:], res[:])

    nc.compile()
    return nc


# ----------------------------------------------------------------------------
# Entry point
# ----------------------------------------------------------------------------

def kernel(x, edge_index, W1, att_src1, att_dst1, b1, W2, att_src2, att_dst2,
           b2, _trace=False):
    x = np.asarray(x, np.float32)
    edge_index = np.asarray(edge_index, np.int64)
    key = "prog"
    if key not in _CACHE:
        cores, w1, B1, W1r, M1 = host_prep(x, edge_index)
        nc = build_program(w1, B1, W1r)
        _CACHE[key] = (nc, cores, M1)
    nc, cores, M1 = _CACHE[key]

    shared = dict(
        M1=M1,
        w1f=np.asarray(W1, np.float32).reshape(1, 256),
        as1=np.tile(np.asarray(att_src1, np.float32).reshape(128), 2)
            .reshape(1, 256),
        ad1=np.tile(np.asarray(att_dst1, np.float32).reshape(128), 2)
            .reshape(1, 256),
        b1=np.asarray(b1, np.float32).reshape(P, 1),
        w2=np.ascontiguousarray(np.asarray(W2, np.float32)),
        w2t=np.ascontiguousarray(np.asarray(W2, np.float32).T),
        att2=np.ascontiguousarray(np.stack(
            [np.asarray(att_src2, np.float32).reshape(128),
             np.asarray(att_dst2, np.float32).reshape(128)], axis=1)),
        b2=np.asarray(b2, np.float32).reshape(1, 128),
        ones=np.ones((1, 128), np.float32),
        ident=np.eye(128, dtype=np.float32),
    )
    # W-hat: Wh[k*4+h, h*32+c] = W1[k, h*32+c]
    W1a = np.asarray(W1, np.float32)
    wh = np.zeros((8, 128), np.float32)
    for h in range(4):
        for k in range(2):
            wh[4 * k + h, h * 32:(h + 1) * 32] = W1a[k, h * 32:(h + 1) * 32]
    shared["wh"] = wh

    in_maps = []
    for c in range(NCORES):
        m = dict(shared)
        m.update(cores[c])
        in_maps.append(m)
    res = run_bass_kernel_spmd(nc, in_maps, core_ids=list(range(NCORES)),
                               trace=_trace)
    out = res.results[0]["out"].reshape(128).astype(np.float32)
    kernel.last_exec_ns = res.exec_time_ns
    return out
